# revision 20
# baseline (speedup 1.0000x reference)
"""MultiBoxLoss Trainium2 Bass kernel.

Data-parallel over the batch: 8 images -> 8 NeuronCores, one image per core.
Each core computes 6 partial scalars; the host combines them into the 3 losses.

Layout: priors are mapped to SBUF [128 partitions, 151 cols] with prior
g = j*128 + p (column-major chunks); 19328 slots, the last 80 are padding.
Jaccard overlaps live in [128, 151*16] "j-major" order (free idx = j*16 + o).
"""

import numpy as np

import concourse.bass as bass
import concourse.bacc as bacc_mod
import concourse.mybir as mybir
import concourse.tile as tile
from concourse.bass import AP, IndirectOffsetOnAxis
from concourse.masks import make_identity

F32 = mybir.dt.float32
I32 = mybir.dt.int32
U32 = mybir.dt.uint32
ALU = mybir.AluOpType
ACTF = mybir.ActivationFunctionType
AX = mybir.AxisListType

P = 19248          # priors
NP = 128           # partitions
L = 151            # cols per partition (128*151 = 19328 >= P)
PADN = NP * L      # 19328
O = 16             # gt objects per image
C = 81             # classes
M2 = 256           # mask_size**2
CB = 12            # compacted slot blocks of 128 -> capacity 1536 positives
CAPF = CB * 8      # 96 cols in the [16, .] compacted layout (16*96 = 1536)
NSLOT = NP * CB    # 1536
JO = L * O         # 2416
TINY = 1e-12


def bc(ap: AP, dims, off: int = 0) -> AP:
    """Replace the free dims of `ap` with explicit [step, count] pairs."""
    import dataclasses
    return dataclasses.replace(
        ap, offset=ap.offset + off,
        ap=[list(ap.ap[0])] + [list(d) for d in dims])


def build_nc() -> bass.Bass:
    nc = bacc_mod.Bacc()

    loc = nc.dram_tensor("loc", [P, 4], F32, kind="ExternalInput")
    conf = nc.dram_tensor("conf", [P, C], F32, kind="ExternalInput")
    mask = nc.dram_tensor("mask", [P, M2], F32, kind="ExternalInput")
    priors = nc.dram_tensor("priors", [P, 4], F32, kind="ExternalInput")
    gtb = nc.dram_tensor("gtb", [O, 4], F32, kind="ExternalInput")
    gtl = nc.dram_tensor("gtl", [O, 1], I32, kind="ExternalInput")
    gtm = nc.dram_tensor("gtm", [O, M2], F32, kind="ExternalInput")
    out = nc.dram_tensor("partials", [1, 8], F32, kind="ExternalOutput")

    with tile.TileContext(nc) as tc:
        with (
            tc.tile_pool(name="const", bufs=1) as cpool,
            tc.tile_pool(name="geo", bufs=1) as geo,
            tc.tile_pool(name="big", bufs=2) as big,
            tc.tile_pool(name="mat", bufs=1) as mat,
            tc.tile_pool(name="conf_s", bufs=3) as confp,
            tc.tile_pool(name="small", bufs=1) as sp,
            tc.tile_pool(name="gath", bufs=1) as gp,
            tc.tile_pool(name="gpipe", bufs=3) as gpipe,
            tc.tile_pool(name="psum", bufs=1, space="PSUM") as pp,
            tc.tile_pool(name="psum1", bufs=1, space="PSUM") as pp1,
        ):
            # ---------------- constants ----------------
            ident = cpool.tile([NP, NP], F32)
            make_identity(nc, ident)
            ones2d = cpool.tile([NP, NP], F32)
            nc.vector.memset(ones2d, 1.0)

            # iota over g = p + 128*j  (int then cast to f32)
            iog_i = cpool.tile([NP, L], I32)
            nc.gpsimd.iota(iog_i, pattern=[[NP, L]], channel_multiplier=1)
            iog = cpool.tile([NP, L], F32)
            nc.vector.tensor_copy(iog, iog_i)
            # valid prior mask [128,151]
            vm = cpool.tile([NP, L], F32)
            nc.vector.tensor_scalar(vm, iog, float(P), None, op0=ALU.is_lt)

            # iota over o: values 0..15, 1+o, and 2*o
            io16_i = cpool.tile([NP, O], I32)
            nc.gpsimd.iota(io16_i, pattern=[[1, O]], channel_multiplier=0)
            io16 = cpool.tile([NP, O], F32)
            nc.vector.tensor_copy(io16, io16_i)
            io16p1 = cpool.tile([NP, O], F32)
            nc.vector.tensor_scalar(io16p1, io16, 1.0, None, op0=ALU.add)
            io16x2 = cpool.tile([NP, O], F32)
            nc.vector.tensor_scalar(io16x2, io16, 2.0, None, op0=ALU.mult)

            # iota over classes 0..80
            io81_i = cpool.tile([NP, C], I32)
            nc.gpsimd.iota(io81_i, pattern=[[1, C]], channel_multiplier=0)
            io81 = cpool.tile([NP, C], F32)
            nc.vector.tensor_copy(io81, io81_i)

            # ---------------- priors / gt geometry ----------------
            # prior planes [128, 151]: cx, cy, w, h  (g = j*128 + p)
            pcx = geo.tile([NP, L], F32)
            pcy = geo.tile([NP, L], F32)
            pw = geo.tile([NP, L], F32)
            ph = geo.tile([NP, L], F32)
            for k, t in enumerate((pcx, pcy, pw, ph)):
                nc.vector.memset(t, 1.0 if k >= 2 else 0.0)
                nc.sync.dma_start(
                    out=t[:, 0:150],
                    in_=priors[: 150 * NP, k].rearrange("(j p) -> p j", p=NP),
                )
                nc.sync.dma_start(
                    out=t[: P - 150 * NP, 150:151],
                    in_=priors[150 * NP :, k].rearrange("(j p) -> p j", p=P - 150 * NP),
                )
            px1 = geo.tile([NP, L], F32)
            px2 = geo.tile([NP, L], F32)
            py1 = geo.tile([NP, L], F32)
            py2 = geo.tile([NP, L], F32)
            # half-sizes
            pw2 = geo.tile([NP, L], F32)
            ph2 = geo.tile([NP, L], F32)
            nc.vector.tensor_scalar(pw2, pw, 0.5, None, op0=ALU.mult)
            nc.vector.tensor_scalar(ph2, ph, 0.5, None, op0=ALU.mult)
            nc.vector.tensor_sub(px1, pcx, pw2)
            nc.vector.tensor_add(px2, pcx, pw2)
            nc.vector.tensor_sub(py1, pcy, ph2)
            nc.vector.tensor_add(py2, pcy, ph2)
            areap = geo.tile([NP, L], F32)
            nc.vector.tensor_mul(areap, pw, ph)

            # gt boxes broadcast to all partitions: [128, 64] (o-major, coord minor)
            gtb_b = geo.tile([NP, O * 4], F32)
            nc.sync.dma_start(
                out=gtb_b,
                in_=AP(tensor=gtb, offset=0, ap=[[0, NP], [1, O * 4]]),
            )
            # per-object area [128,16]
            tw = sp.tile([NP, O], F32)
            th = sp.tile([NP, O], F32)
            areat = geo.tile([NP, O], F32)
            nc.vector.tensor_tensor(
                out=tw, in0=bc(gtb_b[:], [[4, O]], 2),
                in1=bc(gtb_b[:], [[4, O]]), op=ALU.subtract)
            nc.vector.tensor_tensor(
                out=th, in0=bc(gtb_b[:], [[4, O]], 3),
                in1=bc(gtb_b[:], [[4, O]], 1), op=ALU.subtract)
            nc.vector.tensor_mul(areat, tw, th)

            # ---------------- jaccard (j-major [128, 151*16]) ----------------
            def pr(t):   # prior plane broadcast over o
                return bc(t[:], [[1, L], [0, O]])

            def gt(k):   # gt coord k broadcast over j
                return bc(gtb_b[:], [[0, L], [4, O]], k)

            ix1 = big.tile([NP, JO], F32, tag="jac0")
            iy1 = big.tile([NP, JO], F32, tag="jac1")
            ix2 = big.tile([NP, JO], F32, tag="jac2")
            iy2 = big.tile([NP, JO], F32, tag="jac3")
            nc.vector.tensor_tensor(out=ix1, in0=pr(px1), in1=gt(0), op=ALU.max)
            nc.vector.tensor_tensor(out=iy1, in0=pr(py1), in1=gt(1), op=ALU.max)
            nc.vector.tensor_tensor(out=ix2, in0=pr(px2), in1=gt(2), op=ALU.min)
            nc.vector.tensor_tensor(out=iy2, in0=pr(py2), in1=gt(3), op=ALU.min)
            iw = big.tile([NP, JO], F32, tag="jac0")
            ih = big.tile([NP, JO], F32, tag="jac1")
            nc.vector.tensor_sub(iw, ix2, ix1)
            nc.vector.tensor_sub(ih, iy2, iy1)
            iwc = big.tile([NP, JO], F32, tag="jac2")
            ihc = big.tile([NP, JO], F32, tag="jac3")
            nc.vector.tensor_scalar_max(iwc, iw, 0.0)
            nc.vector.tensor_scalar_max(ihc, ih, 0.0)
            inter = big.tile([NP, JO], F32, tag="jac0")
            nc.vector.tensor_mul(inter, iwc, ihc)
            interc = big.tile([NP, JO], F32, tag="jac1")
            nc.vector.tensor_scalar_max(interc, inter, TINY)
            asum = big.tile([NP, JO], F32, tag="jac2")
            nc.vector.tensor_tensor(
                out=asum, in0=bc(areap[:], [[1, L], [0, O]]),
                in1=bc(areat[:], [[0, L], [1, O]]), op=ALU.add)
            # threshold mask: IoU >= 0.5  <=>  3*inter >= A+B
            thr3 = big.tile([NP, JO], F32, tag="jac3")
            nc.vector.scalar_tensor_tensor(
                out=thr3, in0=interc, scalar=3.0, in1=asum, op0=ALU.mult, op1=ALU.is_ge)
            den = big.tile([NP, JO], F32, tag="jac0")  # A+B-inter
            nc.vector.scalar_tensor_tensor(
                out=den, in0=interc, scalar=-1.0, in1=asum, op0=ALU.mult, op1=ALU.add)
            lni = big.tile([NP, JO], F32, tag="jac1")
            nc.scalar.activation(lni, interc, ACTF.Ln)
            lnd = big.tile([NP, JO], F32, tag="jac2")
            nc.scalar.activation(lnd, den, ACTF.Ln)
            llr = big.tile([NP, JO], F32, tag="jac0")
            nc.vector.tensor_sub(llr, lni, lnd)

            # ---------------- matching ----------------
            llr3 = llr[:].rearrange("p (j o) -> p j o", o=O)
            # per-object max over this partition's priors: [128, 16]
            permax = sp.tile([NP, O], F32)
            nc.vector.tensor_reduce(
                out=permax, in_=llr[:].rearrange("p (j o) -> p o j", o=O),
                axis=AX.X, op=ALU.max)
            # cross-partition max -> M[o] as [16,1], then broadcast [128,16]
            permt_ps = pp.tile([O, NP], F32, tag="ps_t")
            nc.tensor.transpose(out=permt_ps, in_=permax[:], identity=ident[:])
            permt = sp.tile([O, NP], F32)
            nc.vector.tensor_copy(permt, permt_ps)
            m16 = sp.tile([O, 1], F32)
            nc.vector.tensor_reduce(out=m16, in_=permt[:], axis=AX.X, op=ALU.max)
            m16t_ps = pp.tile([1, O], F32, tag="ps_r")
            nc.tensor.transpose(out=m16t_ps, in_=m16[:], identity=ident[:O, :O])
            m16t = sp.tile([1, O], F32)
            nc.vector.tensor_copy(m16t, m16t_ps)
            mb_ps = pp.tile([NP, O], F32, tag="ps_b")
            nc.tensor.matmul(out=mb_ps, lhsT=ones2d[:1, :], rhs=m16t[:], start=True, stop=True)
            mb = sp.tile([NP, O], F32)
            nc.vector.tensor_copy(mb, mb_ps)

            # forced: eqM = (llr == M[o]); fplus = max_o eqM*(o+1)
            eqm = big.tile([NP, JO], F32, tag="jac1")
            nc.vector.tensor_tensor(
                out=eqm, in0=llr, in1=bc(mb[:], [[0, L], [1, O]]), op=ALU.is_equal)
            fin = big.tile([NP, JO], F32, tag="jac2")
            nc.vector.tensor_tensor(
                out=fin, in0=eqm, in1=bc(io16p1[:], [[0, L], [1, O]]), op=ALU.mult)
            fplus = mat.tile([NP, L], F32)
            nc.vector.tensor_reduce(
                out=fplus, in_=fin[:].rearrange("p (j o) -> p j o", o=O),
                axis=AX.X, op=ALU.max)

            # per-prior best object
            btmax = mat.tile([NP, L], F32)
            nc.vector.tensor_reduce(out=btmax, in_=llr3, axis=AX.X, op=ALU.max)
            eqb = big.tile([NP, JO], F32, tag="jac3")
            nc.vector.tensor_tensor(
                out=eqb, in0=llr, in1=bc(btmax[:], [[1, L], [0, O]]), op=ALU.is_equal)
            # pack = 2*o + thr  summed over the (unique) argmax position
            tpk = big.tile([NP, JO], F32, tag="jac1")
            nc.vector.tensor_tensor(
                out=tpk, in0=thr3, in1=bc(io16x2[:], [[0, L], [1, O]]), op=ALU.add)
            bsel = big.tile([NP, JO], F32, tag="jac2")
            nc.vector.tensor_mul(bsel, eqb, tpk)
            btpack = mat.tile([NP, L], F32)
            nc.vector.tensor_reduce(
                out=btpack, in_=bsel[:].rearrange("p (j o) -> p j o", o=O),
                axis=AX.X, op=ALU.add)
            # decode via int ops (values are small non-negative ints)
            btp_i = sp.tile([NP, L], I32)
            nc.vector.tensor_copy(btp_i, btpack)
            thr_i = sp.tile([NP, L], I32)
            nc.vector.tensor_scalar(thr_i, btp_i, 1, None, op0=ALU.bitwise_and)
            bto_i = sp.tile([NP, L], I32)
            nc.vector.tensor_scalar(bto_i, btp_i, 1, None, op0=ALU.arith_shift_right)
            thrch = mat.tile([NP, L], F32)
            nc.vector.tensor_copy(thrch, thr_i)
            btidx = mat.tile([NP, L], F32)
            nc.vector.tensor_copy(btidx, bto_i)

            forcedm = mat.tile([NP, L], F32)
            nc.vector.tensor_scalar(forcedm, fplus, 0.5, None, op0=ALU.is_gt)
            fidx = mat.tile([NP, L], F32)
            nc.vector.tensor_scalar(fidx, fplus, 1.0, None, op0=ALU.subtract)
            forced_i = sp.tile([NP, L], I32)
            nc.vector.tensor_copy(forced_i, forcedm)
            o_fin = mat.tile([NP, L], F32)
            nc.vector.tensor_copy(o_fin, btidx)
            nc.vector.copy_predicated(o_fin, forced_i, fidx)
            posf = mat.tile([NP, L], F32)
            nc.vector.tensor_max(posf, thrch, forcedm)
            nc.vector.tensor_mul(posf, posf, vm)
            # per-partition positive count
            posp = sp.tile([NP, 1], F32)
            nc.vector.tensor_reduce(out=posp, in_=posf[:], axis=AX.X, op=ALU.add)
            npos_ps = pp.tile([NP, 1], F32, tag="ps_c")
            nc.tensor.matmul(out=npos_ps, lhsT=ones2d[:], rhs=posp[:], start=True, stop=True)
            kb = sp.tile([NP, 1], F32)
            nc.vector.tensor_scalar(kb, npos_ps, 3.0, float(P - 1), op0=ALU.mult, op1=ALU.min)

            # ---------------- conf stream: lse + conf[:,0] ----------------
            lsesum = mat.tile([NP, L], F32)
            c0 = mat.tile([NP, L], F32)
            G = 10
            for j0 in range(0, L, G):
                g = min(G, L - j0)
                ctile = confp.tile([NP, G * C], F32, tag="conf")
                rows_lo = j0 * NP
                rows_hi = min(P, (j0 + g) * NP)
                if rows_hi - rows_lo == g * NP:
                    nc.sync.dma_start(
                        out=ctile[:, : g * C].rearrange("p (jj c) -> p jj c", c=C),
                        in_=conf[rows_lo:rows_hi, :].rearrange("(jj p) c -> p jj c", p=NP),
                    )
                else:
                    nc.vector.memset(ctile[:, : g * C], 0.0)
                    nrem = rows_hi - rows_lo  # 48 rows in the last column
                    nc.sync.dma_start(
                        out=ctile[:nrem, :C].rearrange("p (jj c) -> p jj c", c=C),
                        in_=conf[rows_lo:rows_hi, :].rearrange("(jj p) c -> p jj c", p=nrem),
                    )
                etile = confp.tile([NP, G * C], F32, tag="exp")
                nc.scalar.activation(etile[:, : g * C], ctile[:, : g * C], ACTF.Exp)
                nc.vector.tensor_reduce(
                    out=lsesum[:, j0 : j0 + g],
                    in_=etile[:, : g * C].rearrange("p (jj c) -> p jj c", c=C),
                    axis=AX.X, op=ALU.add)
                nc.vector.tensor_copy(
                    c0[:, j0 : j0 + g], bc(ctile[:], [[C, g]]))
            lse = mat.tile([NP, L], F32)
            nc.scalar.activation(lse, lsesum, ACTF.Ln)
            # sum of lse over positives
            plse_m = sp.tile([NP, L], F32)
            nc.vector.tensor_mul(plse_m, lse, posf)
            plse = sp.tile([NP, 1], F32)
            nc.vector.tensor_reduce(out=plse, in_=plse_m[:], axis=AX.X, op=ALU.add)

            # mine = (lse - conf0) masked to negatives-and-valid
            mine = mat.tile([NP, L], F32)
            nc.vector.tensor_sub(mine, lse, c0)
            selneg = sp.tile([NP, L], F32)
            nc.vector.tensor_sub(selneg, vm, posf)
            nc.vector.tensor_mul(mine, mine, selneg)

            # ---------------- top-k threshold search ----------------
            mxp = sp.tile([NP, 1], F32)
            nc.vector.tensor_reduce(out=mxp, in_=mine[:], axis=AX.X, op=ALU.max)
            mxt_ps = pp.tile([1, NP], F32, tag="ps_r")
            nc.tensor.transpose(out=mxt_ps, in_=mxp[:], identity=ident[:])
            mxt = sp.tile([1, NP], F32)
            nc.vector.tensor_copy(mxt, mxt_ps)
            mx1 = sp.tile([1, 1], F32)
            nc.vector.tensor_reduce(out=mx1, in_=mxt[:], axis=AX.X, op=ALU.max)
            hi_ps = pp.tile([NP, 1], F32, tag="ps_c")
            nc.tensor.matmul(out=hi_ps, lhsT=ones2d[:1, :], rhs=mx1[:], start=True, stop=True)
            hi = sp.tile([NP, 1], F32)
            nc.vector.tensor_copy(hi, hi_ps)
            lo = sp.tile([NP, 1], F32)
            nc.vector.memset(lo, 0.0)

            NT = 15  # thresholds per round (16-way split)
            for rnd in range(5):
                dd = sp.tile([NP, 1], F32, tag="tk_d")
                nc.vector.tensor_sub(dd, hi, lo)
                th = sp.tile([NP, NT], F32, tag="tk_th")
                for i in range(NT):
                    nc.vector.tensor_scalar(
                        th[:, i : i + 1], dd, (i + 1) / (NT + 1), None, op0=ALU.mult)
                nc.vector.tensor_tensor(
                    out=th, in0=th, in1=bc(lo[:], [[0, NT]]), op=ALU.add)
                cnt = sp.tile([NP, NT], F32, tag="tk_cnt")
                cm = sp.tile([NP, L], F32, tag="tk_cm")
                for i in range(NT):
                    nc.vector.tensor_scalar(
                        cm, mine, th[:, i : i + 1], None, op0=ALU.is_gt,
                        op1=ALU.add, accum_out=cnt[:, i : i + 1])
                cnt_ps = pp.tile([NP, NT], F32, tag="ps_cnt")
                nc.tensor.matmul(out=cnt_ps, lhsT=ones2d[:], rhs=cnt[:], start=True, stop=True)
                sgt = sp.tile([NP, NT], F32, tag="tk_sgt")
                nc.vector.tensor_scalar(sgt, cnt_ps, kb[:], None, op0=ALU.is_gt)
                nsel = sp.tile([NP, 1], F32, tag="tk_nsel")
                nc.vector.tensor_reduce(out=nsel, in_=sgt[:], axis=AX.X, op=ALU.add)
                # lo += d * nsel/(NT+1);  hi = lo + d/(NT+1)
                step = sp.tile([NP, 1], F32, tag="tk_step")
                nc.vector.tensor_scalar(step, dd, 1.0 / (NT + 1), None, op0=ALU.mult)
                dlo = sp.tile([NP, 1], F32, tag="tk_dlo")
                nc.vector.tensor_mul(dlo, step, nsel)
                nc.vector.tensor_add(lo, lo, dlo)
                nc.vector.tensor_add(hi, lo, step)

            # topk sum = sum(relu(mine - t)) + k*t  with t = hi
            srel = sp.tile([NP, 1], F32)
            scr = sp.tile([NP, L], F32, tag="tk_cm")
            nc.vector.tensor_scalar(
                scr, mine, hi[:], 0.0, op0=ALU.subtract, op1=ALU.max)
            nc.vector.tensor_reduce(out=srel, in_=scr[:], axis=AX.X, op=ALU.add)
            kt = sp.tile([NP, 1], F32)
            nc.vector.tensor_mul(kt, kb, hi)

            # ---------------- candidate extraction (max8 x 4 rounds) ----------------
            # score = posf * (16*(4096-j) + o): distinct per column, decodable
            CK = 32
            val16_i = cpool.tile([NP, L], I32)
            nc.gpsimd.iota(val16_i, pattern=[[-16, L]], base=65536, channel_multiplier=0)
            val16 = cpool.tile([NP, L], F32)
            nc.vector.tensor_copy(val16, val16_i)
            iop_i = cpool.tile([NP, 1], I32)
            nc.gpsimd.iota(iop_i, pattern=[[1, 1]], channel_multiplier=1)
            score = mat.tile([NP, L], F32)
            nc.vector.tensor_add(score, val16, o_fin)
            nc.vector.tensor_mul(score, score, posf)
            cand = gp.tile([NP, CK], F32)
            sc_cur = score
            for r in range(4):
                nc.vector.max(out=cand[:, r * 8 : (r + 1) * 8], in_=sc_cur[:])
                if r < 3:
                    sc_nxt = confp.tile([NP, L], F32, tag="scmr")
                    nc.vector.match_replace(
                        out=sc_nxt, in_to_replace=cand[:, r * 8 : (r + 1) * 8],
                        in_values=sc_cur[:], imm_value=0.0)
                    sc_cur = sc_nxt
            vslot = gp.tile([NP, CK], F32)
            nc.vector.tensor_scalar(vslot, cand, 0.5, None, op0=ALU.is_gt)
            cand_i = gp.tile([NP, CK], I32)
            nc.vector.tensor_copy(cand_i, cand)
            o_i = gp.tile([NP, CK], I32)
            nc.vector.tensor_scalar(o_i, cand_i, 15, None, op0=ALU.bitwise_and)
            o_self = gp.tile([NP, CK], F32)
            nc.vector.tensor_copy(o_self, o_i)
            jv_i = gp.tile([NP, CK], I32)
            nc.vector.tensor_scalar(jv_i, cand_i, 4, None, op0=ALU.arith_shift_right)
            jvf = gp.tile([NP, CK], F32)
            nc.vector.tensor_copy(jvf, jv_i)
            iopf = cpool.tile([NP, 1], F32)
            nc.vector.tensor_copy(iopf, iop_i)
            gf = gp.tile([NP, CK], F32)
            nc.vector.tensor_scalar(gf, jvf, -128.0, 4096.0 * 128.0, op0=ALU.mult, op1=ALU.add)
            nc.vector.tensor_scalar(gf, gf, iopf[:, :1], float(P - 1), op0=ALU.add, op1=ALU.min)
            g_i = gp.tile([NP, CK], I32)
            nc.vector.tensor_copy(g_i, gf)

            # ---------------- gathers: one indirect call per candidate column ------
            mk_g = gp.tile([NP, CK * M2], F32)
            cf_g = gp.tile([NP, CK * C], F32)
            for k in range(CK):
                nc.gpsimd.indirect_dma_start(
                    out=mk_g[:, k * M2 : (k + 1) * M2], out_offset=None,
                    in_=mask[:, :],
                    in_offset=IndirectOffsetOnAxis(ap=g_i[:, k : k + 1], axis=0))
                nc.gpsimd.indirect_dma_start(
                    out=cf_g[:, k * C : (k + 1) * C], out_offset=None,
                    in_=conf[:, :],
                    in_offset=IndirectOffsetOnAxis(ap=g_i[:, k : k + 1], axis=0))

            # ---------------- mask BCE via per-object PE accumulation -------------
            # L1[o,m] = sum_{pos slots of o} ln(1-p); L2[o,m] = sum ln(p)
            gtm_sb = geo.tile([O, M2], F32)
            nc.sync.dma_start(out=gtm_sb, in_=gtm[:, :])
            l1ps = pp1.tile([O, M2], F32, tag="ps_l1")
            l2ps = pp1.tile([O, M2], F32, tag="ps_l2")
            for pas, (ps_t, sc_ln, bi_ln) in enumerate(
                    ((l1ps, -1.0, 1.0), (l2ps, 1.0, 0.0))):
                for k in range(CK):
                    ohk = gpipe.tile([NP, O], F32, tag="ohk")
                    nc.vector.tensor_scalar(
                        ohk, io16, o_self[:, k : k + 1], vslot[:, k : k + 1],
                        op0=ALU.is_equal, op1=ALU.mult)
                    lnk = gpipe.tile([NP, M2], F32, tag="lnk")
                    nc.scalar.activation(
                        lnk, mk_g[:, k * M2 : (k + 1) * M2], ACTF.Ln,
                        bias=bi_ln, scale=sc_ln)
                    nc.tensor.matmul(out=ps_t, lhsT=ohk[:], rhs=lnk[:],
                                     start=(k == 0), stop=(k == CK - 1))
            l1sb = sp.tile([O, M2], F32)
            nc.vector.tensor_copy(l1sb, l1ps)
            dd16 = sp.tile([O, M2], F32)
            nc.vector.tensor_sub(dd16, l2ps, l1sb)
            nc.vector.tensor_mul(dd16, dd16, gtm_sb)
            ff16 = sp.tile([O, M2], F32)
            nc.vector.tensor_add(ff16, dd16, l1sb)
            sbce16 = sp.tile([O, 1], F32)
            nc.vector.tensor_reduce(out=sbce16, in_=ff16[:], axis=AX.X, op=ALU.add)

            # ---------------- gt-class conf on candidate slots --------------------
            lab_b = geo.tile([NP, O], I32)
            nc.sync.dma_start(
                out=lab_b, in_=AP(tensor=gtl, offset=0, ap=[[0, NP], [1, O]]))
            clsmap = geo.tile([NP, O], F32)
            nc.vector.tensor_scalar(clsmap, lab_b, 1.0, None, op0=ALU.add)
            ohall = gp.tile([NP, CK * O], F32)
            nc.vector.tensor_tensor(
                out=ohall[:].rearrange("p (k o) -> p k o", o=O),
                in0=bc(io16[:], [[0, CK], [1, O]]),
                in1=bc(o_self[:], [[1, CK], [0, O]]), op=ALU.is_equal)
            nc.vector.tensor_tensor(
                out=ohall[:].rearrange("p (k o) -> p k o", o=O),
                in0=ohall[:].rearrange("p (k o) -> p k o", o=O),
                in1=bc(clsmap[:], [[0, CK], [1, O]]), op=ALU.mult)
            clsall = gp.tile([NP, CK], F32)
            nc.vector.tensor_reduce(
                out=clsall, in_=ohall[:].rearrange("p (k o) -> p k o", o=O),
                axis=AX.X, op=ALU.add)
            ohc = gp.tile([NP, CK * C], F32)
            nc.vector.tensor_tensor(
                out=ohc[:].rearrange("p (k c) -> p k c", c=C),
                in0=bc(io81[:], [[0, CK], [1, C]]),
                in1=bc(clsall[:], [[1, CK], [0, C]]), op=ALU.is_equal)
            nc.vector.tensor_mul(ohc, ohc, cf_g)
            zsl = gp.tile([NP, CK], F32)
            nc.vector.tensor_reduce(
                out=zsl, in_=ohc[:].rearrange("p (k c) -> p k c", c=C),
                axis=AX.X, op=ALU.add)
            nc.vector.tensor_mul(zsl, zsl, vslot)
            zsum = sp.tile([NP, 1], F32)
            nc.vector.tensor_reduce(out=zsum, in_=zsl[:], axis=AX.X, op=ALU.add)

            # ---------------- smooth-L1 loc loss (dense) --------------------------
            lx = []
            for k in range(4):
                t = mat.tile([NP, L], F32, tag=f"locp{k}")
                nc.vector.memset(t, 0.0)
                nc.sync.dma_start(
                    out=t[:, 0:150],
                    in_=loc[: 150 * NP, k].rearrange("(j p) -> p j", p=NP))
                nc.sync.dma_start(
                    out=t[: P - 150 * NP, 150:151],
                    in_=loc[150 * NP :, k].rearrange("(j p) -> p j", p=P - 150 * NP))
                lx.append(t)
            oh2 = big.tile([NP, JO], F32, tag="jac3")
            nc.vector.tensor_tensor(
                out=oh2, in0=bc(o_fin[:], [[1, L], [0, O]]),
                in1=bc(io16[:], [[0, L], [1, O]]), op=ALU.is_equal)
            tb = []
            for k in range(4):
                selk = big.tile([NP, JO], F32, tag="jac1")
                nc.vector.tensor_tensor(out=selk, in0=oh2, in1=gt(k), op=ALU.mult)
                tbk = mat.tile([NP, L], F32, tag=f"tb{k}")
                nc.vector.tensor_reduce(
                    out=tbk, in_=selk[:].rearrange("p (j o) -> p j o", o=O),
                    axis=AX.X, op=ALU.add)
                tb.append(tbk)
            rpw = mat.tile([NP, L], F32, tag="rpw")
            rph = mat.tile([NP, L], F32, tag="rph")
            nc.vector.reciprocal(rpw, pw[:])
            nc.vector.reciprocal(rph, ph[:])
            slsum = sp.tile([NP, 1], F32)
            nc.vector.memset(slsum, 0.0)
            st1 = mat.tile([NP, L], F32, tag="st1")
            st2 = mat.tile([NP, L], F32, tag="st2")
            for c in range(4):
                if c < 2:  # g_cx, g_cy
                    nc.vector.tensor_add(st1, tb[c], tb[c + 2])
                    nc.vector.tensor_scalar(st1, st1, 0.5, None, op0=ALU.mult)
                    nc.vector.tensor_sub(st1, st1, (pcx, pcy)[c])
                    nc.vector.tensor_scalar(st1, st1, 10.0, None, op0=ALU.mult)
                    nc.vector.tensor_mul(st1, st1, (rpw, rph)[c])
                else:      # g_w, g_h
                    nc.vector.tensor_sub(st1, tb[c], tb[c - 2])
                    nc.vector.tensor_mul(st1, st1, (rpw, rph)[c - 2])
                    nc.scalar.activation(st2, st1, ACTF.Ln)
                    nc.vector.tensor_scalar(st1, st2, 5.0, None, op0=ALU.mult)
                nc.vector.tensor_sub(st1, lx[c], st1)          # d
                nc.vector.scalar_tensor_tensor(
                    out=st2, in0=st1, scalar=-1.0, in1=st1, op0=ALU.mult, op1=ALU.max)
                nc.vector.tensor_scalar_min(st1, st2, 1.0)     # m = min(|d|,1)
                nc.vector.scalar_tensor_tensor(
                    out=st2, in0=st1, scalar=-0.5, in1=st2, op0=ALU.mult, op1=ALU.add)
                nc.vector.tensor_mul(st1, st1, st2)            # m*(|d|-m/2)
                nc.vector.tensor_mul(st1, st1, posf)
                pc = sp.tile([NP, 1], F32, tag="slpart")
                nc.vector.tensor_reduce(out=pc, in_=st1[:], axis=AX.X, op=ALU.add)
                nc.vector.tensor_add(slsum, slsum, pc)

            # ---------------- final assembly ----------------
            stack = sp.tile([NP, 8], F32)
            nc.vector.memset(stack, 0.0)
            nc.vector.tensor_copy(stack[:, 0:1], srel)   # sum relu(mine - t)
            nc.vector.tensor_copy(stack[:, 1:2], plse)   # sum_pos lse
            nc.vector.tensor_copy(stack[:, 2:3], zsum)   # sum_pos conf[gt class]
            nc.vector.tensor_copy(stack[:O, 3:4], sbce16)
            nc.vector.tensor_copy(stack[:, 4:5], slsum)  # loc loss
            nc.vector.tensor_copy(stack[:, 5:6], posp)   # positives count
            nc.vector.tensor_copy(stack[:, 6:7], kt)     # k*t (same on all partitions)
            nc.vector.tensor_copy(stack[:, 7:8], kb)     # k (same on all partitions)
            fin_ps = pp1.tile([1, 8], F32)
            nc.tensor.matmul(out=fin_ps, lhsT=ones2d[:, :1], rhs=stack[:], start=True, stop=True)
            outsb = sp.tile([1, 8], F32)
            nc.vector.tensor_copy(outsb, fin_ps)
            nc.sync.dma_start(out=out[:, :], in_=outsb[:])

    nc.compile()
    return nc


_NC_CACHE = None


def _get_nc():
    global _NC_CACHE
    if _NC_CACHE is None:
        _NC_CACHE = build_nc()
    return _NC_CACHE


def combine_partials(partials_list):
    """partials_list: list of 8 arrays [1,8] -> full [3] output."""
    sl = sc = sm = n = 0.0
    for v in partials_list:
        v = np.asarray(v, np.float64).reshape(8)
        srel, plse, zsum, sbce, slsum, npos, kt128, _k128 = v
        kt = kt128 / NP
        sc += srel + kt + plse - zsum
        sm += -sbce
        sl += slsum
        n += npos
    out = np.array([sl / n, sc / n, sm / (n * M2) * 100.0 / n], np.float32)
    return out


def kernel(loc_data, conf_data, mask_data, priors, gt_boxes, gt_labels, gt_masks):
    from concourse.bass_utils import run_bass_kernel_spmd

    nc = _get_nc()
    B = loc_data.shape[0]
    in_maps = []
    for b in range(B):
        in_maps.append({
            "loc": np.ascontiguousarray(loc_data[b], np.float32),
            "conf": np.ascontiguousarray(conf_data[b], np.float32),
            "mask": np.ascontiguousarray(mask_data[b], np.float32),
            "priors": np.ascontiguousarray(priors, np.float32),
            "gtb": np.ascontiguousarray(gt_boxes[b], np.float32),
            "gtl": np.ascontiguousarray(gt_labels[b].reshape(O, 1), np.int32),
            "gtm": np.ascontiguousarray(gt_masks[b], np.float32),
        })
    res = run_bass_kernel_spmd(nc, in_maps, core_ids=list(range(B)))
    return combine_partials([r["partials"] for r in res.results])
